# revision 23
# baseline (speedup 1.0000x reference)
"""Trainium2 Bass kernel for nn_CortexReasoner (masked-update attention with
Iron RoPE + relative Fourier bias).

Sharding: one attention head per NeuronCore (n_head == n_cores == 8), both
batches on every core.  The output projection is redistributed with TWO
AllToAlls (one per batch) so the first one overlaps with the second batch's
attention compute; each core finalizes a 256-token slice of each batch.

All heavy operands travel as bf16 (PE runs bf16 matmuls at 1 cycle/row vs 2
for fp32r, and DMA bytes halve); accumulation stays fp32 in PSUM.
"""

import math
import os
import sys

import numpy as np
import ml_dtypes

for _p in ("/opt/trn_rl_repo",):
    if _p not in sys.path and os.path.isdir(_p):
        sys.path.append(_p)

import concourse.bass as bass
import concourse.mybir as mybir
import concourse.tile as tile
from concourse.bass_utils import run_bass_kernel_spmd

F32 = mybir.dt.float32
BF16 = mybir.dt.bfloat16
AF = mybir.ActivationFunctionType

B, T, D = 2, 2048, 1024
H = 8
HD = 128          # head dim
N_CORES = 8
BT = B * T        # 4096
ROWS = BT // N_CORES   # 512 output rows per core (256 from each batch)
NCH = 8           # t-chunks of 512 across B*T
CT = D // 128     # 8 contraction tiles for the projections
KT = T // 128     # 16 key tiles per batch
QC = T // 512     # 4 query chunks per batch
OWN = T // N_CORES     # 256 tokens owned per batch per core


def _build_nc():
    nc = bass.Bass()

    xT = nc.dram_tensor("xT", [D, BT], BF16, kind="ExternalInput")
    wq = nc.dram_tensor("wq", [128, CT * HD], BF16, kind="ExternalInput")
    wk = nc.dram_tensor("wk", [128, CT * HD], BF16, kind="ExternalInput")
    wv = nc.dram_tensor("wv", [128, CT * HD], BF16, kind="ExternalInput")
    bq = nc.dram_tensor("bq", [HD, 1], F32, kind="ExternalInput")
    bk = nc.dram_tensor("bk", [HD, 1], F32, kind="ExternalInput")
    bv = nc.dram_tensor("bv", [HD, 1], F32, kind="ExternalInput")
    At = nc.dram_tensor("At", [B, 128, T], BF16, kind="ExternalInput")    # [cos;cos]
    Bt = nc.dram_tensor("Bt", [B, 128, T], BF16, kind="ExternalInput")    # [-sin;sin]
    fk = nc.dram_tensor("fk", [B, 64, T], BF16, kind="ExternalInput")
    fq = nc.dram_tensor("fq", [B, 64, T], BF16, kind="ExternalInput")
    pswp = nc.dram_tensor("pswp", [128, 128], BF16, kind="ExternalInput")
    identity = nc.dram_tensor("identity", [128, 128], BF16, kind="ExternalInput")
    onesq = nc.dram_tensor("onesq", [128, 128], BF16, kind="ExternalInput")
    wo = nc.dram_tensor("wo", [128, CT * D], BF16, kind="ExternalInput")
    maskc = nc.dram_tensor("maskc", [ROWS, 1], F32, kind="ExternalInput")
    in1m = nc.dram_tensor("in1m", [ROWS, D], F32, kind="ExternalInput")

    out = nc.dram_tensor("out", [ROWS, D], F32, kind="ExternalOutput")

    with tile.TileContext(nc) as tc, \
         nc.allow_low_precision(reason="bf16 matmul pipeline"):
        with tc.tile_pool(name="persist", bufs=1) as pp, \
             tc.tile_pool(name="consts", bufs=1) as cp, \
             tc.tile_pool(name="dram", bufs=1, space="DRAM") as dp:

            qrot = [pp.tile([128, T], BF16, tag=f"qrot{b}", name=f"qrot{b}") for b in range(B)]
            krot = [pp.tile([128, T], BF16, tag=f"krot{b}", name=f"krot{b}") for b in range(B)]
            vnat = [pp.tile([128, KT * 128], BF16, tag=f"vnat{b}", name=f"vnat{b}") for b in range(B)]

            tP = cp.tile([128, 128], BF16)
            ident = cp.tile([128, 128], BF16)
            tones = cp.tile([128, 128], BF16)
            nc.sync.dma_start(out=tP[:], in_=pswp[:])
            nc.sync.dma_start(out=ident[:], in_=identity[:])
            nc.sync.dma_start(out=tones[:], in_=onesq[:])

            # collective staging (bf16 payloads), one pair per batch
            a2a_in = [dp.tile([N_CORES, 128, OWN], BF16, tag=f"a2ai{b}", name=f"a2ai{b}")
                      for b in range(B)]
            a2a_out = [dp.tile([N_CORES, 128, OWN], BF16, tag=f"a2ao{b}", name=f"a2ao{b}")
                       for b in range(B)]

            # deferred DMA issue schedule: chunk index -> list of thunks.
            # Critical-path-first: weights + chunk-0 x + batch-0 rope tables go
            # ahead of everything else; phase-2/3 operands stream in behind.
            tAt = [None] * B
            tBt = [None] * B
            tfk = [None] * B
            tfq = [None] * B
            two = cp.tile([128, CT * D], BF16)
            tmask = cp.tile([128, 4], F32)
            tin1 = cp.tile([128, 4 * D], F32)

            def load_rope(b):
                def _f():
                    a_b = cp.tile([128, T], BF16, tag=f"At{b}")
                    b_b = cp.tile([128, T], BF16, tag=f"Bt{b}")
                    nc.sync.dma_start(out=a_b[:], in_=At[b])
                    nc.sync.dma_start(out=b_b[:], in_=Bt[b])
                    tAt[b] = a_b
                    tBt[b] = b_b
                return _f

            def load_fourier(b):
                def _f():
                    fkb = cp.tile([64, T], BF16, tag=f"fk{b}")
                    fqb = cp.tile([64, T], BF16, tag=f"fq{b}")
                    nc.sync.dma_start(out=fkb[:], in_=fk[b])
                    nc.sync.dma_start(out=fqb[:], in_=fq[b])
                    tfk[b] = fkb
                    tfq[b] = fqb
                return _f

            def load_wo(half):
                def _f():
                    s = slice(half * CT * D // 2, (half + 1) * CT * D // 2)
                    nc.sync.dma_start(out=two[:, s], in_=wo[:, s])
                return _f

            def load_phase3_misc():
                nc.sync.dma_start(out=tmask[:], in_=maskc.rearrange("(tt p) one -> p (tt one)", p=128))
                for tt in range(4):
                    nc.sync.dma_start(out=tin1[:, tt * D:(tt + 1) * D],
                                      in_=in1m[tt * 128:(tt + 1) * 128, :])

            deferred = {
                0: [load_rope(0)],
                2: [load_rope(1)],
                4: [load_fourier(0)],
                5: [load_fourier(1)],
                6: [load_wo(0), load_wo(1)],
                7: [load_phase3_misc],
            }

            # ---------------- Phase 1: QKV projection + RoPE + V transpose
            with tc.tile_pool(name="ph1", bufs=1) as p1, \
                 tc.tile_pool(name="ph1x", bufs=16) as p1x, \
                 tc.tile_pool(name="ph1s", bufs=4) as p1s, \
                 tc.tile_pool(name="ph1t", bufs=4) as p1t, \
                 tc.tile_pool(name="ps1", bufs=4, space="PSUM") as ps1, \
                 tc.tile_pool(name="ps1b", bufs=2, space="PSUM") as ps1b, \
                 tc.tile_pool(name="ps1c", bufs=2, space="PSUM") as ps1c:

                tbq = p1.tile([128, 1], F32)
                tbk = p1.tile([128, 1], F32)
                tbv = p1.tile([128, 1], F32)
                nc.sync.dma_start(out=tbq[:], in_=bq[:])
                nc.sync.dma_start(out=tbk[:], in_=bk[:])
                nc.sync.dma_start(out=tbv[:], in_=bv[:])
                wqt = p1.tile([128, CT * HD], BF16)
                wkt = p1.tile([128, CT * HD], BF16)
                wvt = p1.tile([128, CT * HD], BF16)
                for hh in range(2):
                    s = slice(hh * CT * HD // 2, (hh + 1) * CT * HD // 2)
                    nc.sync.dma_start(out=wqt[:, s], in_=wq[:, s])
                    nc.sync.dma_start(out=wkt[:, s], in_=wk[:, s])
                    nc.sync.dma_start(out=wvt[:, s], in_=wv[:, s])

                for ch in range(NCH):
                    b = ch // (NCH // B)
                    tch = slice(ch * 512, (ch + 1) * 512)
                    tch_b = slice((ch % 4) * 512, (ch % 4 + 1) * 512)
                    xts = []
                    for ct in range(CT):
                        xt = p1x.tile([128, 512], BF16, tag="xt")
                        nc.sync.dma_start(out=xt[:], in_=xT[ct * 128:(ct + 1) * 128, tch])
                        xts.append(xt)
                    for fn in deferred.get(ch, []):
                        fn()
                    pq = ps1.tile([128, 512], F32, tag="pqkv")
                    pk = ps1.tile([128, 512], F32, tag="pqkv")
                    pv = ps1.tile([128, 512], F32, tag="pqkv")
                    for ct in range(CT):
                        st, sp = (ct == 0), (ct == CT - 1)
                        s = slice(ct * HD, (ct + 1) * HD)
                        nc.tensor.matmul(pq[:], wqt[:, s], xts[ct][:], start=st, stop=sp)
                        nc.tensor.matmul(pk[:], wkt[:, s], xts[ct][:], start=st, stop=sp)
                        nc.tensor.matmul(pv[:], wvt[:, s], xts[ct][:], start=st, stop=sp)

                    # q/k: add bias, rope-rotate into qrot/krot.  The two
                    # swap matmuls are adjacent so the tP weights load once.
                    sq = p1s.tile([128, 512], BF16, tag="sqk")
                    sk = p1s.tile([128, 512], BF16, tag="sqk")
                    nc.scalar.activation(sq[:], pq[:], AF.Identity, bias=tbq[:])
                    nc.scalar.activation(sk[:], pk[:], AF.Identity, bias=tbk[:])
                    pswq = ps1b.tile([128, 512], F32, tag="psw")
                    pswk = ps1b.tile([128, 512], F32, tag="psw")
                    nc.tensor.matmul(pswq[:], tP[:], sq[:], start=True, stop=True)
                    nc.tensor.matmul(pswk[:], tP[:], sk[:], start=True, stop=True)
                    for (sqk, psw, dstl) in ((sq, pswq, qrot), (sk, pswk, krot)):
                        dst = dstl[b]
                        ta = p1t.tile([128, 512], BF16, tag="ropeA")
                        tbm = p1t.tile([128, 512], BF16, tag="ropeB")
                        nc.vector.tensor_mul(ta[:], sqk[:], tAt[b][:, tch_b])
                        nc.vector.tensor_mul(tbm[:], psw[:], tBt[b][:, tch_b])
                        nc.vector.tensor_add(dst[:, tch_b], ta[:], tbm[:])

                    # v: bias then transpose 4x 128x128 into vnat
                    sv = p1s.tile([128, 512], BF16, tag="sv")
                    nc.scalar.activation(sv[:], pv[:], AF.Identity, bias=tbv[:])
                    for j in range(4):
                        ptr = ps1c.tile([128, 128], BF16, tag="ptr")
                        nc.tensor.transpose(ptr[:], sv[:, j * 128:(j + 1) * 128], ident[:])
                        g = (ch % 4) * 4 + j
                        nc.scalar.activation(vnat[b][:, g * 128:(g + 1) * 128], ptr[:], AF.Copy)

            # ---------------- Phase 2: attention, one A2A per batch
            with tc.tile_pool(name="ph2e", bufs=6) as p2e, \
                 tc.tile_pool(name="ph2r", bufs=2) as p2r, \
                 tc.tile_pool(name="ps2", bufs=4, space="PSUM") as ps2, \
                 tc.tile_pool(name="ps2y", bufs=2, space="PSUM") as ps2y, \
                 tc.tile_pool(name="ps2s", bufs=2, space="PSUM") as ps2s:

                # phase-3 input tiles, DMA-triggered early: round 0's loads
                # fire mid-way through batch-1 attention (A2A#0 long done, so
                # the SP queue never blocks), round 1's right after the second
                # collective so they fire the moment its semaphore posts.
                ya_tiles = [[cp.tile([128, OWN], BF16, tag=f"ya{r}_{dt}", name=f"ya{r}_{dt}")
                             for dt in range(N_CORES)] for r in range(B)]

                for u in range(B):
                    for hf in range(2):
                        if u == 1 and hf == 1:
                            for dt in range(N_CORES):
                                nc.sync.dma_start(out=ya_tiles[0][dt][:], in_=a2a_out[0][dt])
                        # two query chunks processed together so every lhsT
                        # (krot / fourier / ones / vnat block) loads once for
                        # two matmuls
                        qs = [slice((2 * hf + j) * 512, (2 * hf + j + 1) * 512)
                              for j in range(2)]
                        py = [ps2y.tile([128, 512], F32, tag="py", name=f"py{u}_{hf}_{j}") for j in range(2)]
                        psm = [ps2s.tile([128, 512], F32, tag="psm", name=f"psm{u}_{hf}_{j}") for j in range(2)]
                        ses = {}
                        for kt in range(KT + 2):
                            if kt < KT:
                                ks = slice(kt * 128, (kt + 1) * 128)
                                psc = [ps2.tile([128, 512], F32, tag="psc", name=f"psc{u}_{hf}_{kt}_{j}") for j in range(2)]
                                se = [p2e.tile([128, 512], BF16, tag="exp", name=f"se{u}_{hf}_{kt}_{j}") for j in range(2)]
                                ses[kt] = se
                                nc.tensor.matmul(psc[0][:], krot[u][:, ks], qrot[u][:, qs[0]], start=True, stop=False)
                                nc.tensor.matmul(psc[1][:], krot[u][:, ks], qrot[u][:, qs[1]], start=True, stop=False)
                                nc.tensor.matmul(psc[0][:], tfk[u][:, ks], tfq[u][:, qs[0]], start=False, stop=True)
                                nc.tensor.matmul(psc[1][:], tfk[u][:, ks], tfq[u][:, qs[1]], start=False, stop=True)
                                nc.scalar.activation(se[0][:], psc[0][:], AF.Exp)
                                nc.scalar.activation(se[1][:], psc[1][:], AF.Exp)
                            if kt >= 2:
                                ktp = kt - 2
                                se = ses.pop(ktp)
                                st, sp = (ktp == 0), (ktp == KT - 1)
                                vs = slice(ktp * 128, (ktp + 1) * 128)
                                nc.tensor.matmul(psm[0][:], tones[:], se[0][:], start=st, stop=sp)
                                nc.tensor.matmul(psm[1][:], tones[:], se[1][:], start=st, stop=sp)
                                nc.tensor.matmul(py[0][:], vnat[u][:, vs], se[0][:], start=st, stop=sp)
                                nc.tensor.matmul(py[1][:], vnat[u][:, vs], se[1][:], start=st, stop=sp)
                        # normalize in 256-wide halves so the reciprocal, the
                        # multiply and the a2a staging DMA pipeline
                        for j in range(2):
                            for half in range(2):
                                hs = slice(half * OWN, (half + 1) * OWN)
                                rbs = p2r.tile([128, OWN], F32, tag="rbs")
                                nc.vector.reciprocal(rbs[:], psm[j][:, hs])
                                ynrm = p2r.tile([128, OWN], BF16, tag="ynrm")
                                nc.vector.tensor_mul(ynrm[:], py[j][:, hs], rbs[:])
                                nc.sync.dma_start(out=a2a_in[u][2 * (2 * hf + j) + half],
                                                  in_=ynrm[:])
                    nc.gpsimd.collective_compute(
                        "AllToAll", mybir.AluOpType.bypass,
                        ins=[a2a_in[u].opt()], outs=[a2a_out[u].opt()],
                        replica_groups=[list(range(N_CORES))],
                    )
                    if u == 1:
                        for dt in range(N_CORES):
                            nc.sync.dma_start(out=ya_tiles[1][dt][:], in_=a2a_out[1][dt])

            # ---------------- Phase 3: output projection per A2A round
            with tc.tile_pool(name="ph3", bufs=1) as p3, \
                 tc.tile_pool(name="ph3s", bufs=4) as p3s, \
                 tc.tile_pool(name="ps3", bufs=4, space="PSUM") as ps3:

                for r in range(B):
                    ya = ya_tiles[r]
                    for tl in range(2):
                        tt = r * 2 + tl
                        po = [ps3.tile([128, 512], F32, tag="po", name=f"po{r}_{tl}_{j}") for j in range(2)]
                        for dt in range(CT):
                            st, sp = (dt == 0), (dt == CT - 1)
                            for nch in range(2):
                                nc.tensor.matmul(po[nch][:], ya[dt][:, tl * 128:(tl + 1) * 128],
                                                 two[:, dt * D + nch * 512: dt * D + (nch + 1) * 512],
                                                 start=st, stop=sp)
                        for nch in range(2):
                            so = p3s.tile([128, 512], F32, tag="so")
                            nc.vector.scalar_tensor_tensor(
                                out=so[:], in0=po[nch][:], scalar=tmask[:, tt:tt + 1],
                                in1=tin1[:, tt * D + nch * 512: tt * D + (nch + 1) * 512],
                                op0=mybir.AluOpType.mult, op1=mybir.AluOpType.add)
                            nc.sync.dma_start(out=out[tt * 128:(tt + 1) * 128, nch * 512:(nch + 1) * 512], in_=so[:])

    _split_multi_waits(nc)
    return nc


def _split_multi_waits(nc):
    """This walrus build encodes at most one sync-wait per instruction; hoist
    extras onto preceding NoOps.  For the kernel-tail drain (many DMA-queue
    waits, followed by an all-engine barrier) spread the NoOps across all
    engines so the waits poll in parallel; elsewhere keep them on the same
    engine to preserve ordering semantics."""
    engs = [mybir.EngineType.SP, mybir.EngineType.Activation, mybir.EngineType.DVE,
            mybir.EngineType.PE, mybir.EngineType.Pool]
    for f in nc.m.functions:
        for bb in f.blocks:
            new_insts = []
            for inst in bb.instructions:
                si = inst.sync_info
                if si is not None and si.on_wait and len(si.on_wait) > 1:
                    waits = list(si.on_wait)
                    distribute = (type(inst).__name__ == "InstDrain"
                                  and len(waits) > 3)
                    for j, w in enumerate(waits[:-1]):
                        eng = engs[j % len(engs)] if distribute else inst.engine
                        new_insts.append(mybir.InstNoOp(
                            name=f"{inst.name}_wsplit{j}", ins=[], outs=[],
                            engine=eng,
                            sync_info=mybir.SyncInfo(on_wait=[w], on_update=[])))
                    si.on_wait = [waits[-1]]
                new_insts.append(inst)
            bb.instructions = new_insts


def _prep_inputs(x, coords, update_mask, Wqkv, bqkv, Wo, bo, W_rope, W_fb,
                 beta_cos, beta_sin):
    """Per-core input maps (host-side layout + tiny trig tables)."""
    f32 = np.float32
    bf16 = ml_dtypes.bfloat16
    x = np.asarray(x, f32)
    coords = np.asarray(coords, f32)
    update_mask = np.asarray(update_mask)
    Wqkv = np.asarray(Wqkv, f32)
    bqkv = np.asarray(bqkv, f32)
    Wo = np.ascontiguousarray(np.asarray(Wo, f32))
    bo = np.asarray(bo, f32)
    W_rope = np.asarray(W_rope, f32)
    W_fb = np.asarray(W_fb, f32)
    beta_cos = np.asarray(beta_cos, f32)
    beta_sin = np.asarray(beta_sin, f32)

    xf = x.reshape(BT, D)
    xT = np.ascontiguousarray(xf.T.astype(bf16))

    # split-half channel order: evens then odds
    perm = np.concatenate([np.arange(0, HD, 2), np.arange(1, HD, 2)])
    inv_scale = f32(1.0 / math.sqrt(HD))

    # rope tables per batch: theta[m, t]; A=[cos;cos], B=[-sin;sin]
    At = np.empty((B, 128, T), bf16)
    Bt = np.empty((B, 128, T), bf16)
    fkT = np.empty((B, 64, T), bf16)
    fqT = np.empty((B, 64, T), bf16)
    for b in range(B):
        c1 = coords[b, :, 0].astype(np.float64)
        th = (W_rope[:, 0:1].astype(np.float64) * c1[None, :])
        cth = np.cos(th).astype(f32)
        sth = np.sin(th).astype(f32)
        At[b] = np.concatenate([cth, cth], axis=0).astype(bf16)
        Bt[b] = np.concatenate([-sth, sth], axis=0).astype(bf16)
        S = (W_fb[:, 0:1].astype(np.float64) * c1[None, :])
        cS = np.cos(S).astype(f32)
        sS = np.sin(S).astype(f32)
        fkT[b] = np.concatenate([cS, sS], axis=0).astype(bf16)
        fqT[b] = np.concatenate([cS * beta_cos[:, None] + sS * beta_sin[:, None],
                                 sS * beta_cos[:, None] - cS * beta_sin[:, None]],
                                axis=0).astype(bf16)

    pswp = np.zeros((128, 128), bf16)
    for i in range(128):
        pswp[(i + 64) % 128, i] = 1.0   # lhsT: (P^T x)[i] = x[(i+64)%128]

    ident_np = np.eye(128, dtype=bf16)
    onesq = np.ones((128, 128), bf16)

    # wo packed: block dt holds Wo rows [dt*128:(dt+1)*128, :]
    wo_pack = np.empty((128, CT * D), bf16)
    for dt in range(CT):
        wo_pack[:, dt * D:(dt + 1) * D] = Wo[dt * 128:(dt + 1) * 128, :].astype(bf16)

    mask_f = update_mask.reshape(BT).astype(f32)

    in_maps = []
    for c in range(N_CORES):
        h = c
        wq_h = (Wqkv[:, h * HD:(h + 1) * HD][:, perm] * inv_scale)
        wk_h = Wqkv[:, D + h * HD:D + (h + 1) * HD][:, perm]
        wv_h = Wqkv[:, 2 * D + h * HD:2 * D + (h + 1) * HD]
        # pack [128, CT*HD]: block ct = w[ct*128:(ct+1)*128, :]
        def pack(w):
            p = np.empty((128, CT * HD), bf16)
            for ct in range(CT):
                p[:, ct * HD:(ct + 1) * HD] = w[ct * 128:(ct + 1) * 128, :].astype(bf16)
            return p
        bq_h = (bqkv[h * HD:(h + 1) * HD][perm] * inv_scale).reshape(HD, 1)
        bk_h = bqkv[D + h * HD:D + (h + 1) * HD][perm].reshape(HD, 1)
        bv_h = bqkv[2 * D + h * HD:2 * D + (h + 1) * HD].reshape(HD, 1)
        # owned rows: batch-0 tokens [c*OWN:(c+1)*OWN] then batch-1 same slice
        rows = np.r_[np.arange(c * OWN, (c + 1) * OWN),
                     np.arange(T + c * OWN, T + (c + 1) * OWN)]
        mrows = mask_f[rows].reshape(ROWS, 1)
        in1 = mrows * bo[None, :] + (1.0 - mrows) * xf[rows]
        in_maps.append(dict(
            xT=xT, wq=pack(wq_h), wk=pack(wk_h), wv=pack(wv_h),
            bq=np.ascontiguousarray(bq_h, f32), bk=np.ascontiguousarray(bk_h, f32),
            bv=np.ascontiguousarray(bv_h, f32),
            At=At, Bt=Bt, fk=fkT, fq=fqT, pswp=pswp,
            identity=ident_np, onesq=onesq, wo=wo_pack,
            maskc=np.ascontiguousarray(mrows, f32),
            in1m=np.ascontiguousarray(in1, f32),
        ))
    return in_maps


_NC_CACHE = None


def _get_nc():
    global _NC_CACHE
    if _NC_CACHE is None:
        _NC_CACHE = _build_nc()
    return _NC_CACHE


def run(trace=False, **inputs):
    nc = _get_nc()
    in_maps = _prep_inputs(**inputs)
    res = run_bass_kernel_spmd(nc, in_maps, core_ids=list(range(N_CORES)),
                               trace=trace)
    full = np.empty((B, T, D), np.float32)
    for c in range(N_CORES):
        o = res.results[c]["out"]
        full[0, c * OWN:(c + 1) * OWN] = o[0:OWN]
        full[1, c * OWN:(c + 1) * OWN] = o[OWN:ROWS]
    return full, res


def kernel(**inputs) -> np.ndarray:
    full, _ = run(trace=False, **inputs)
    return full
